# revision 19
# baseline (speedup 1.0000x reference)
"""Trainium2 Bass kernel for nn_Attention_5334349382130.

Module: y = softmax((x@Wq+bq)(x@Wk+bk)^T / d^2) (x@Wv+bv) @ Wo + bo
  with B=4, N=4096, C=256, 4 heads of dim 64, scale = 1/4096.

Sharding (8 cores): core c handles batch b=c//2 and head-pair hp=c%2
(global heads {2hp, 2hp+1} = inner-dim columns hp*128 .. hp*128+128).
Each core computes its two heads' attention plus the partial output
projection over its 128 rows of Wo. Host sums the two partials per batch
and adds bo + bv@Wo (softmax rows sum to 1, so V's bias contributes
exactly bv@Wo).

Key numerics: scores s = (q.k)/4096 satisfy |s| < 0.005 for this input
distribution (checked empirically, randn x with uniform +-1/16 weights),
so softmax needs no max-subtraction and fp16 matmul inputs keep relative
error ~1e-4 level. All PSUM accumulation is fp32.

Per-core device pipeline:
  A) load x [4096,256] f32, PE-transpose to xT f16 [c=2x128, n=4096]
  B) projections: qT,kT = Wq/Wk^T-layout matmuls (+bias, f16),
     V natural layout packed as Vaug[h] = [V_h | ones] [m, 128]
  C) per head, per 1024-wide query block:
       for each of 32 key tiles: S'[m128, 1024] = K_h Q_h^T (PSUM),
       P = exp(S'/4096) via ACT (f16, scale fused),
       O[128,1024] += Vaug_h^T @ P  (rows 0-63 = O^T, 64-127 = denom)
     normalize: Ocat^T[h*64:, :] = O[0:64] * recip(O[64:128])
     out-proj: Y[n128, 256] = Ocat^T_slice.T @ Wo_local, DMA out.
"""

import os
import sys

for _p in ("/root/.axon_site/_ro/trn_rl_repo", "/opt/trn_rl_repo"):
    if os.path.isdir(_p) and _p not in sys.path:
        sys.path.append(_p)

import numpy as np

B, N, C = 4, 4096, 256
NUM_HEADS, DIM_HEAD = 4, 64
SCALE = 1.0 / (DIM_HEAD * DIM_HEAD)
P = 128
NB = 1024          # query-block width
NBLK = N // NB     # 4 query blocks
MT = N // P        # 32 key tiles
NT = N // P        # 32 row tiles

_last_results = None
_nc_cache = None


def _build():
    import concourse.bass as bass
    import concourse.mybir as mybir
    import concourse.tile as tile
    from concourse import bacc

    f32 = mybir.dt.float32
    f16 = mybir.dt.float16
    Exp = mybir.ActivationFunctionType.Exp
    Identity = mybir.ActivationFunctionType.Identity
    mult = mybir.AluOpType.mult

    nc = bacc.Bacc("TRN2", target_bir_lowering=False, debug=False)

    x_in = nc.dram_tensor("x", (N, C), f32, kind="ExternalInput").ap()
    wq_in = nc.dram_tensor("wq", (C, P), f32, kind="ExternalInput").ap()
    wk_in = nc.dram_tensor("wk", (C, P), f32, kind="ExternalInput").ap()
    wv_in = nc.dram_tensor("wv", (C, P), f32, kind="ExternalInput").ap()
    wo_in = nc.dram_tensor("wo", (P, C), f32, kind="ExternalInput").ap()
    bq_in = nc.dram_tensor("bq", (P,), f32, kind="ExternalInput").ap()
    bk_in = nc.dram_tensor("bk", (P,), f32, kind="ExternalInput").ap()
    y_out = nc.dram_tensor("y", (N, C), f32, kind="ExternalOutput").ap()

    CH = C // P  # 2 contraction tiles over c

    with tile.TileContext(nc) as tc:
        with tc.tile_pool(name="const", bufs=1) as const, \
             tc.tile_pool(name="big", bufs=1) as big, \
             tc.tile_pool(name="dram", bufs=1, space="DRAM") as dram:
            # --- constants ---
            def load_w(ap_in, shape3, nm):
                t32 = const.tile(list(shape3), f32, tag="wstage", name=f"stage_{nm}")
                nc.sync.dma_start(t32[:], ap_in)
                t16 = const.tile(list(shape3), f16, tag=nm, name=nm)
                nc.vector.tensor_copy(t16[:], t32[:])
                return t16

            wq16 = load_w(wq_in.rearrange("(kt p) m -> p kt m", p=P), (P, CH, P), "wq16")
            wk16 = load_w(wk_in.rearrange("(kt p) m -> p kt m", p=P), (P, CH, P), "wk16")
            wv16 = load_w(wv_in.rearrange("(kt p) m -> p kt m", p=P), (P, CH, P), "wv16")
            wo16 = load_w(wo_in, (P, C), "wo16")

            bq_sb = const.tile([P, 1], f32)
            bk_sb = const.tile([P, 1], f32)
            with nc.allow_non_contiguous_dma(reason="128x4B bias column load"):
                nc.sync.dma_start(bq_sb[:], bq_in[:, None])
                nc.sync.dma_start(bk_sb[:], bk_in[:, None])

            # warm the ACT exp table set early (one-time ~2.7us load)
            warm = const.tile([P, 1], f32)
            nc.scalar.activation(warm[:], bq_sb[:], Exp, scale=0.0)

            # --- big persistent SBUF tensors ---
            x_sb = big.tile([P, NT, C], f32)      # x staged f32, n on partitions
            x16 = big.tile([P, NT, C], f16)       # x cast to f16
            xT = big.tile([P, CH, N], f16)        # x^T, c on partitions
            qT = big.tile([P, N], f16)            # Q^T, 2 heads stacked
            kT = big.tile([P, N], f16)
            # vcat[:, mt]: [V_h0 | ones | V_h1 | ones] (64 cols each) so
            # mm2 lhsT for head h is the contiguous slice [h*128 : h*128+128]
            # = [V_h | ones] -> out rows 0:64 = O^T, 64:128 = denominators.
            vcat = big.tile([P, MT, 2 * P], f16)
            xh = dram.tile([N, C], f16)           # DRAM scratch for DMA-transpose

            # ============ Phase A+B: load, cast, DMA-transpose, project ====
            vcat4 = vcat[:].rearrange("p m (a c) -> p m a c", a=2)
            nc.vector.memset(vcat4[:, :, :, DIM_HEAD:], 1.0)
            x_r = x_in.rearrange("(nt p) c -> p nt c", p=P)
            xh_r = xh[:].rearrange("(nt p) c -> p nt c", p=P)
            NCHUNK = 4
            TPC = NT // NCHUNK  # 8 n-tiles per chunk
            RPC = TPC * P       # 1024 x-rows per chunk
            for cchunk in range(NCHUNK):
                t0 = cchunk * TPC
                nc.sync.dma_start(x_sb[:, t0:t0 + TPC, :], x_r[:, t0:t0 + TPC, :])
            with tc.tile_pool(name="ppsum", bufs=2, space="PSUM") as ppsum, \
                 tc.tile_pool(name="vpsum", bufs=2, space="PSUM") as vpsum:
                for cchunk in range(NCHUNK):
                    t0 = cchunk * TPC
                    r0 = cchunk * RPC
                    nc.vector.tensor_copy(x16[:, t0:t0 + TPC, :], x_sb[:, t0:t0 + TPC, :])
                    nc.scalar.dma_start(xh_r[:, t0:t0 + TPC, :], x16[:, t0:t0 + TPC, :])
                    for ch in range(CH):
                        nc.sync.dma_start_transpose(
                            xT[:, ch, r0:r0 + RPC],
                            xh[:][r0:r0 + RPC, ch * P:(ch + 1) * P])
                    for blk in range(cchunk * 2, cchunk * 2 + 2):
                        ps = ppsum.tile([P, 512], f32, tag="proj", name="kps")
                        for ch in range(CH):
                            nc.tensor.matmul(ps[:], lhsT=wk16[:, ch, :],
                                             rhs=xT[:, ch, blk * 512:(blk + 1) * 512],
                                             start=(ch == 0), stop=(ch == CH - 1))
                        nc.vector.tensor_scalar_add(kT[:, blk * 512:(blk + 1) * 512],
                                                    ps[:], bk_sb[:])
                        ps = ppsum.tile([P, 512], f32, tag="proj", name="qps")
                        for ch in range(CH):
                            nc.tensor.matmul(ps[:], lhsT=wq16[:, ch, :],
                                             rhs=xT[:, ch, blk * 512:(blk + 1) * 512],
                                             start=(ch == 0), stop=(ch == CH - 1))
                        nc.vector.tensor_scalar_add(qT[:, blk * 512:(blk + 1) * 512],
                                                    ps[:], bq_sb[:])
                    for mt in range(t0, t0 + TPC):
                        ps = vpsum.tile([P, P], f32, tag="vproj", name="vps")
                        for ch in range(CH):
                            nc.tensor.matmul(ps[:], lhsT=xT[:, ch, mt * P:(mt + 1) * P],
                                             rhs=wv16[:, ch, :],
                                             start=(ch == 0), stop=(ch == CH - 1))
                        nc.vector.tensor_copy(
                            vcat4[:, mt, :, :DIM_HEAD],
                            ps[:].rearrange("p (a c) -> p a c", a=2))

            # ============ Phase C: attention + out-proj ============
            # Phase C: per 512-query block, both heads together. The two mm1s
            # use disjoint PE row groups (K=64 at partition base 0 and 64) so
            # they run concurrently; one exp instr covers both heads' scores.
            QB = 512
            with tc.tile_pool(name="spsum", bufs=2, space="PSUM") as spsum, \
                 tc.tile_pool(name="opsum", bufs=4, space="PSUM") as opsum, \
                 tc.tile_pool(name="pexp", bufs=4) as pexp, \
                 tc.tile_pool(name="onorm", bufs=2) as onorm, \
                 tc.tile_pool(name="rnorm", bufs=2) as rnorm, \
                 tc.tile_pool(name="ystage", bufs=3) as ystage:
                for blk in range(N // QB):
                    qs = slice(blk * QB, (blk + 1) * QB)
                    o_ps = [opsum.tile([P, QB], f32, tag="oacc", name=f"o_ps{h}")
                            for h in range(2)]
                    for mt in range(MT):
                        s_ps = spsum.tile([P, 2 * QB], f32)
                        for h in range(2):
                            hs = slice(h * DIM_HEAD, (h + 1) * DIM_HEAD)
                            nc.tensor.matmul(
                                s_ps[:, h * QB:(h + 1) * QB],
                                lhsT=kT[hs, mt * P:(mt + 1) * P],
                                rhs=qT[hs, qs], start=True, stop=True)
                        p_sb = pexp.tile([P, 2 * QB], f16)
                        nc.scalar.activation(p_sb[:], s_ps[:], Exp, scale=SCALE)
                        for h in range(2):
                            nc.tensor.matmul(
                                o_ps[h][:], lhsT=vcat[:, mt, h * P:(h + 1) * P],
                                rhs=p_sb[:, h * QB:(h + 1) * QB],
                                start=(mt == 0), stop=(mt == MT - 1))
                    osb = onorm.tile([P, QB], f16)
                    for h in range(2):
                        hs = slice(h * DIM_HEAD, (h + 1) * DIM_HEAD)
                        rec = rnorm.tile([DIM_HEAD, QB], f32, tag="rec", name="rec")
                        nc.vector.reciprocal(rec[:], o_ps[h][DIM_HEAD:, :])
                        nc.vector.tensor_tensor(osb[hs, :], o_ps[h][:DIM_HEAD, :],
                                                rec[:], mult)
                    for t in range(QB // P):
                        y_ps = opsum.tile([P, C], f32, tag="oacc", name="y_ps")
                        nc.tensor.matmul(y_ps[:], lhsT=osb[:, t * P:(t + 1) * P],
                                         rhs=wo16[:], start=True, stop=True)
                        y_sb = ystage.tile([P, C], f32)
                        nc.vector.tensor_copy(y_sb[:], y_ps[:])
                        nc.sync.dma_start(
                            y_out[(blk * (QB // P) + t) * P:(blk * (QB // P) + t + 1) * P, :],
                            y_sb[:])
    nc.compile()
    return nc


def kernel(x, Wq, bq, Wk, bk, Wv, bv, Wo, bo):
    global _last_results, _nc_cache
    from concourse import bass_utils

    x = np.ascontiguousarray(np.asarray(x, dtype=np.float32))
    Wq = np.asarray(Wq, dtype=np.float32)
    bq = np.asarray(bq, dtype=np.float32)
    Wk = np.asarray(Wk, dtype=np.float32)
    bk = np.asarray(bk, dtype=np.float32)
    Wv = np.asarray(Wv, dtype=np.float32)
    bv = np.asarray(bv, dtype=np.float32)
    Wo = np.asarray(Wo, dtype=np.float32)
    bo = np.asarray(bo, dtype=np.float32)

    if _nc_cache is None:
        _nc_cache = _build()
    nc = _nc_cache

    in_maps = []
    for c in range(8):
        b, hp = c // 2, c % 2
        js = slice(hp * P, hp * P + P)
        in_maps.append({
            "x": np.ascontiguousarray(x[b]),
            "wq": np.ascontiguousarray(Wq[:, js]),
            "wk": np.ascontiguousarray(Wk[:, js]),
            "wv": np.ascontiguousarray(Wv[:, js]),
            "wo": np.ascontiguousarray(Wo[js, :]),
            "bq": np.ascontiguousarray(bq[js]),
            "bk": np.ascontiguousarray(bk[js]),
        })

    br = bass_utils.run_bass_kernel_spmd(nc, in_maps, core_ids=list(range(8)))
    _last_results = br

    ypart = np.stack([r["y"] for r in br.results])          # [8, N, C]
    const_row = bv @ Wo + bo                                 # [C], exact fp32
    out = ypart[0::2] + ypart[1::2] + const_row[None, None, :]
    return out.astype(np.float32)
